# revision 21
# baseline (speedup 1.0000x reference)
"""DemandMap (histogram_binning) Trainium2 Bass kernel — packed-pair encode.

Math (binW=binH=1, integer sites, sx=1): per row r, along c:
  cap1[c] = m1[c];  cap2[c] = m2[c] + m2[c-1] + 0.5 m2[c-2];
  cap3[c] = sum_{s<5} m3[c-s];  out_t = 1 - cap_t (out0 == out1).

Per site the device computes E = c2x2 + 6*c3 + 36*g1 (c2x2 = 2*cap2 via
taps (2,2,1) on y2 = m2; c3 via 6*ones(5) taps on y3x = m3 + 6*m1 whose
m1 rider gives g1 = 5-tap m1 sum; base-6 fields, E <= 215). Each fp8
DoubleRow matmul (k-slot 0 = W2 band on y2, k-slot 1 = W3 band on y3x;
0.5 cyc/row, K-independent) handles ONE row parity; even and odd rows
accumulate into the same PSUM column with the odd WEIGHTS scaled 2^-8
(per-matmul sums stay same-scale -> reduction tree exact; the f32 PSUM
accumulator adds E_even + E_odd/256 exactly). One evacuation per chunk
converts x256 - 32768 to int16 = 256*E_even + E_odd - 32768: two sites
per evacuated element, so evac free-size halves and stores stay 1 B/site.

Host decode: +32768 -> (hi, lo) = (E_even, E_odd); c2x2 = E%6 and
c3 = (E//6)%6 are the device rasterizations; m1 (1x1 sites, cap1 is
just the mask) via a stride-5 cumsum of diff(g1 = E//36).

Column-sharded: 2 x 124-col column-major tiles per core + an 8-col
row-major bf16 mini path (5 DVE ops). PE is kept continuously busy with
scratch warmup matmuls so real matmuls run at full clock. Stores issue
from three different sequencers (SP/ACT/DVE) to overlap their waits.
"""

from contextlib import ExitStack

import numpy as np
import ml_dtypes

import concourse.bass as bass
import concourse.mybir as mybir
from concourse.bass_utils import run_bass_kernel_spmd

N_CORES = 8
R = 2048              # rows
HP = 1024             # row pairs
CP = 256              # output columns per core
P = 128               # partitions
OC = 124              # output columns per main tile
KC = 512              # matmul chunk (one PSUM bank; 512 row-pairs)

_A = mybir.AluOpType
BF = mybir.dt.bfloat16
FP8 = mybir.dt.float8e4
I8 = mybir.dt.int8
I16 = mybir.dt.int16
F32 = mybir.dt.float32
Copy = mybir.ActivationFunctionType.Copy
DR = mybir.MatmulPerfMode.DoubleRow

NWARM, NGAP1, NGAP2 = 29, 54, 40    # PE warmup / gap-filler matmuls

LAST_RESULTS = None


def _build_program():
    nc = bass.Bass()
    # [P, slot(y2/y3x), 128 Weven | 128 Wodd | 1024 even | 1024 odd]
    ydw0d = nc.dram_tensor("ydw0", [P, 2, 2 * P + 2 * HP], FP8,
                           kind="ExternalInput")
    yd1d = nc.dram_tensor("yd1", [P, 2, 2, HP], FP8, kind="ExternalInput")
    ymd = nc.dram_tensor("ym", [P, 3, 16, 12], BF, kind="ExternalInput")
    e0d = nc.dram_tensor("e0", [OC, HP], I16, kind="ExternalOutput")
    e1d = nc.dram_tensor("e1", [OC, HP], I16, kind="ExternalOutput")
    emd = nc.dram_tensor("em", [P, 16, 8], I8, kind="ExternalOutput")

    with ExitStack() as ctx:
        sb = lambda nm, shape, dt: ctx.enter_context(nc.sbuf_tensor(nm, shape, dt))
        ydw0 = sb("ydw0s", [P, 2, 2 * P + 2 * HP], FP8)
        ys1 = sb("ys1s", [P, 2, 2, HP], FP8)
        yms = sb("ymsb", [P, 3, 16, 12], BF)
        es = [sb(f"es{t}", [P, HP], I16) for t in range(2)]
        ems = sb("emsb", [P, 16, 8], I8)
        mt = [sb(f"mt{i}", [P, 16, 8], BF) for i in range(2)]
        scr = sb("scr", [P, 2, 256], FP8)
        ps = [ctx.enter_context(nc.psum_tensor(f"ps{i}", [P, KC], F32))
              for i in range(4)]
        psd = ctx.enter_context(nc.psum_tensor("psd", [P, 256], F32))

        sem = lambda nm: ctx.enter_context(nc.semaphore(nm))
        sin0, sym = sem("sin0"), sem("sym")
        sin1a, sin1b = sem("sin1a"), sem("sin1b")
        spsA, spsB = sem("spsA"), sem("spsB")
        sev0, sevA, sevB = sem("sev0"), sem("sevA"), sem("sevB")
        smini, sscr = sem("smini"), sem("sscr")
        st_sp = sem("st_sp")
        st_gp = sem("st_gp")
        block = ctx.enter_context(nc.Block())

        we = ydw0[:, :, 0:OC]
        wo = ydw0[:, :, P:P + OC]
        t0e = lambda c: ydw0[:, :, 2 * P + c * KC:2 * P + (c + 1) * KC]
        t0o = lambda c: ydw0[:, :, 2 * P + HP + c * KC:2 * P + HP + (c + 1) * KC]

        @block.sync
        def _(sync):
            sync.dma_start(out=ydw0[:], in_=ydw0d[:]).then_inc(sin0, 16)
            sync.dma_start(out=yms[:], in_=ymd[:]).then_inc(sym, 16)
            sync.dma_start(out=ys1[:, :, :, 0:KC],
                           in_=yd1d[:, :, :, 0:KC]).then_inc(sin1a, 16)
            sync.dma_start(out=ys1[:, :, :, KC:HP],
                           in_=yd1d[:, :, :, KC:HP]).then_inc(sin1b, 16)
            sync.dma_start(out=e1d[:, KC:HP], in_=es[1][0:OC, KC:HP])._wait_ge(
                sevB, 1).then_inc(st_sp, 16)
            sync.wait_ge(st_sp, 32)

        @block.tensor
        def _(pe):
            dummy = lambda: pe.matmul(psd[0:P, :], scr[:, :, 0:P],
                                      scr[:, :, 0:256], start=True, stop=True,
                                      perf_mode=DR)
            small = lambda: pe.matmul(psd[0:32, 0:64], scr[:, :, 0:32],
                                      scr[:, :, 0:64], start=True, stop=True,
                                      perf_mode=DR)
            dummy()._wait_ge(sscr, 1)
            for _ in range(NWARM - 1):
                dummy()
            pe.wait_ge(sin0, 16)
            for c in range(2):
                pe.matmul(ps[c][0:OC, :], we, t0e(c), start=True,
                          stop=False, perf_mode=DR)
                pe.matmul(ps[c][0:OC, :], wo, t0o(c), start=False, stop=True,
                          perf_mode=DR).then_inc(spsA, 1)
            for _ in range(NGAP1):
                small()
            pe.wait_ge(sin1a, 16)
            pe.matmul(ps[2][0:OC, :], we, ys1[:, :, 0, 0:KC], start=True,
                      stop=False, perf_mode=DR)
            pe.matmul(ps[2][0:OC, :], wo, ys1[:, :, 1, 0:KC], start=False,
                      stop=True, perf_mode=DR).then_inc(spsB, 1)
            for _ in range(NGAP2):
                small()
            pe.wait_ge(sin1b, 16)
            pe.matmul(ps[3][0:OC, :], we, ys1[:, :, 0, KC:HP], start=True,
                      stop=False, perf_mode=DR)
            pe.matmul(ps[3][0:OC, :], wo, ys1[:, :, 1, KC:HP], start=False,
                      stop=True, perf_mode=DR).then_inc(spsB, 1)

        @block.scalar
        def _(act):
            act.activation(es[0][0:OC, 0:KC], ps[0][0:OC, :], Copy,
                           bias=-32768.0, scale=256.0)._wait_ge(
                spsA, 1).then_inc(sev0, 1)
            act.activation(es[1][0:OC, 0:KC], ps[2][0:OC, :], Copy,
                           bias=-32768.0, scale=256.0)._wait_ge(
                spsB, 1).then_inc(sevA, 1)
            act.dma_start(out=e1d[:, 0:KC], in_=es[1][0:OC, 0:KC])._wait_ge(
                sevA, 1).then_inc(st_sp, 16)

        @block.vector
        def _(v):
            v.memset(scr[:], 0).then_inc(sscr, 1)
            Pm, Qm, Rm = yms[:, 0], yms[:, 1], yms[:, 2]
            # mini: E = P[j]+P[j-1]+Q[j-2]+R[j-3]+R[j-4] - 128
            v.tensor_tensor(mt[0][:], Pm[:, :, 4:12], Pm[:, :, 3:11],
                            _A.add)._wait_ge(sym, 16)
            v.tensor_tensor(mt[1][:], Qm[:, :, 2:10], Rm[:, :, 1:9], _A.add)
            v.tensor_tensor(mt[0][:], mt[0][:], Rm[:, :, 0:8], _A.add)
            v.tensor_tensor(mt[0][:], mt[0][:], mt[1][:], _A.add)
            v.tensor_scalar(ems[:], mt[0][:], -128.0, None,
                            _A.add).then_inc(smini, 1)
            # evacuations c1 of each tile
            v.tensor_scalar(es[0][0:OC, KC:HP], ps[1][0:OC, :], 256.0,
                            -32768.0, _A.mult, _A.add)._wait_ge(
                spsA, 2).then_inc(sev0, 1)
            v.tensor_scalar(es[1][0:OC, KC:HP], ps[3][0:OC, :],
                            256.0, -32768.0, _A.mult, _A.add)._wait_ge(
                spsB, 2).then_inc(sevB, 1)

        @block.gpsimd
        def _(gp):
            gp.dma_start(out=emd[:], in_=ems[:])._wait_ge(
                smini, 1).then_inc(st_gp, 16)
            gp.dma_start(out=e0d[:], in_=es[0][0:OC, :])._wait_ge(
                sev0, 2).then_inc(st_gp, 16)
            gp.wait_ge(st_gp, 32)

    return nc


def _host_inputs(stm):
    f8 = ml_dtypes.float8_e4m3
    bf = ml_dtypes.bfloat16
    stm2d = np.asarray(stm, dtype=np.int32).reshape(R, R)      # [r, c]
    T = np.ascontiguousarray(stm2d.T)                          # [c, r]
    y2 = (T == 2).astype(np.float32)
    y3x = ((T == 3) + 6.0 * (T == 1)).astype(np.float32)
    # split row parities: [c, parity, HP]
    y2p = y2.reshape(R, HP, 2).transpose(0, 2, 1)
    y3p = y3x.reshape(R, HP, 2).transpose(0, 2, 1)
    padz = np.zeros((4, 2, HP), np.float32)
    y2p = np.concatenate([padz, y2p], axis=0).astype(f8)       # c index +4
    y3p = np.concatenate([padz, y3p], axis=0).astype(f8)

    w = np.zeros((P, 2, 2 * P), np.float32)
    for j in range(OC):
        w[j + 4, 0, j] = 2.0                # W2 band: taps (2, 2, 1)
        w[j + 3, 0, j] = 2.0
        w[j + 2, 0, j] = 1.0
        w[j:j + 5, 1, j] = 6.0              # W3 band: 6 * ones(5)
    w[:, :, P:] = w[:, :, 0:P] * (1.0 / 256.0)                 # odd weights
    wf8 = w.astype(f8)

    in_maps = []
    for k in range(N_CORES):
        B = CP * k
        # ydw0 data region: per slot, even columns then odd columns
        s0 = np.concatenate([y2p[B:B + P, 0], y2p[B:B + P, 1]], axis=-1)
        s1 = np.concatenate([y3p[B:B + P, 0], y3p[B:B + P, 1]], axis=-1)
        t0 = np.stack([s0, s1], axis=1)                        # [P, 2, 2HP]
        ydw0 = np.concatenate([wf8, t0], axis=2)               # [P, 2, 2304]
        t1 = np.stack([y2p[B + OC:B + OC + P],
                       y3p[B + OC:B + OC + P]], axis=1)        # [P, 2, 2, HP]
        # mini: columns B+244..B+256 (4 halo + 8 out), row-major
        sl = stm2d[:, B + 244:B + 256]                         # [r, 12]
        m2 = (sl == 2).astype(np.float32)
        y3 = ((sl == 3) + 6.0 * (sl == 1)).astype(np.float32)
        Pw, Qw, Rw = 2 * m2 + 6 * y3, m2 + 6 * y3, 6 * y3
        ym = np.stack([Pw, Qw, Rw], axis=1)                    # [r, 3, 12]
        ym = ym.reshape(16, P, 3, 12).transpose(1, 2, 0, 3)    # [P,3,16,12]
        in_maps.append({
            "ydw0": np.ascontiguousarray(ydw0),
            "yd1": np.ascontiguousarray(t1),
            "ym": np.ascontiguousarray(ym.astype(bf)),
        })
    return in_maps


def kernel(site_type_map, node_size_x, node_size_y, width, height,
           num_bins_x, num_bins_y, xl, xh, yl, yh):
    global LAST_RESULTS
    nc = _build_program()
    in_maps = _host_inputs(site_type_map)
    res = run_bass_kernel_spmd(nc, in_maps, core_ids=list(range(N_CORES)))
    LAST_RESULTS = res

    ET = np.empty((R, R), np.int16)        # [c, r], E in 0..215
    for k in range(N_CORES):
        B = CP * k
        rk = res.results[k]
        for t, nm in ((0, "e0"), (1, "e1")):
            e = np.asarray(rk[nm]).astype(np.int32) + 32768    # [OC, HP]
            blk = ET[B + t * OC:B + (t + 1) * OC]
            blk[:, 0::2] = e >> 8
            blk[:, 1::2] = e & 255
        em = np.asarray(rk["em"]).astype(np.int16) + 128       # [P, 16, 8]
        ET[B + 248:B + 256] = em.transpose(1, 0, 2).reshape(R, 8).T

    c2x2 = ET % 6
    c3 = (ET // 6) % 6
    g1 = (ET // 36).astype(np.int8)
    # m1[c] = g1[c] - g1[c-1] + m1[c-5]: stride-5 cumsum along c of diff(g1)
    d = np.empty((R + 2, R), np.int8)      # pad c-length 2048 -> 2050
    d[0] = g1[0]
    np.subtract(g1[1:], g1[:-1], out=d[1:R])
    d[R:] = 0
    m1 = np.cumsum(d.reshape(410, 5, R), axis=0, dtype=np.int16)
    m1 = m1.reshape(R + 2, R)[:R]

    out0 = np.ascontiguousarray((1.0 - m1).astype(np.float32).T)
    out2 = np.ascontiguousarray((1.0 - 0.5 * c2x2).astype(np.float32).T)
    out3 = np.ascontiguousarray((1.0 - c3).astype(np.float32).T)
    return (out0, out0, out2, out3)


# revision 22
# speedup vs baseline: 1.0027x; 1.0027x over previous
"""DemandMap (histogram_binning) Trainium2 Bass kernel — packed-pair encode.

Math (binW=binH=1, integer sites, sx=1): per row r, along c:
  cap1[c] = m1[c];  cap2[c] = m2[c] + m2[c-1] + 0.5 m2[c-2];
  cap3[c] = sum_{s<5} m3[c-s];  out_t = 1 - cap_t (out0 == out1).

Per site the device computes E = c2x2 + 6*c3 + 36*g1 (c2x2 = 2*cap2 via
taps (2,2,1) on y2 = m2; c3 via 6*ones(5) taps on y3x = m3 + 6*m1 whose
m1 rider gives g1 = 5-tap m1 sum; base-6 fields, E <= 215). Each fp8
DoubleRow matmul (k-slot 0 = W2 band on y2, k-slot 1 = W3 band on y3x;
0.5 cyc/row, K-independent) handles ONE row parity; even and odd rows
accumulate into the same PSUM column with the odd WEIGHTS scaled 2^-8
(per-matmul sums stay same-scale -> reduction tree exact; the f32 PSUM
accumulator adds E_even + E_odd/256 exactly). One evacuation per chunk
converts x256 - 32768 to int16 = 256*E_even + E_odd - 32768: two sites
per evacuated element, so evac free-size halves and stores stay 1 B/site.

Host decode: +32768 -> (hi, lo) = (E_even, E_odd); c2x2 = E%6 and
c3 = (E//6)%6 are the device rasterizations; m1 (1x1 sites, cap1 is
just the mask) via a stride-5 cumsum of diff(g1 = E//36).

Column-sharded: 2 x 124-col column-major tiles per core + an 8-col
row-major bf16 mini path (5 DVE ops). PE is kept continuously busy with
scratch warmup matmuls so real matmuls run at full clock. Stores issue
from three different sequencers (SP/ACT/DVE) to overlap their waits.
"""

from contextlib import ExitStack

import numpy as np
import ml_dtypes

import concourse.bass as bass
import concourse.mybir as mybir
from concourse.bass_utils import run_bass_kernel_spmd

N_CORES = 8
R = 2048              # rows
HP = 1024             # row pairs
CP = 256              # output columns per core
P = 128               # partitions
OC = 124              # output columns per main tile
KC = 512              # matmul chunk (one PSUM bank; 512 row-pairs)

_A = mybir.AluOpType
BF = mybir.dt.bfloat16
FP8 = mybir.dt.float8e4
I8 = mybir.dt.int8
I16 = mybir.dt.int16
F32 = mybir.dt.float32
Copy = mybir.ActivationFunctionType.Copy
DR = mybir.MatmulPerfMode.DoubleRow

NWARM, NGAP1, NGAP2 = 29, 49, 35    # PE warmup / gap-filler matmuls

LAST_RESULTS = None


def _build_program():
    nc = bass.Bass()
    # [P, slot(y2/y3x), 128 Weven | 128 Wodd | 1024 even | 1024 odd]
    ydw0d = nc.dram_tensor("ydw0", [P, 2, 2 * P + 2 * HP], FP8,
                           kind="ExternalInput")
    yd1d = nc.dram_tensor("yd1", [P, 2, 2, HP], FP8, kind="ExternalInput")
    ymd = nc.dram_tensor("ym", [P, 3, 16, 12], BF, kind="ExternalInput")
    e0d = nc.dram_tensor("e0", [OC, HP], I16, kind="ExternalOutput")
    e1d = nc.dram_tensor("e1", [OC, HP], I16, kind="ExternalOutput")
    emd = nc.dram_tensor("em", [P, 16, 8], I8, kind="ExternalOutput")

    with ExitStack() as ctx:
        sb = lambda nm, shape, dt: ctx.enter_context(nc.sbuf_tensor(nm, shape, dt))
        ydw0 = sb("ydw0s", [P, 2, 2 * P + 2 * HP], FP8)
        ys1 = sb("ys1s", [P, 2, 2, HP], FP8)
        yms = sb("ymsb", [P, 3, 16, 12], BF)
        es = [sb(f"es{t}", [P, HP], I16) for t in range(2)]
        ems = sb("emsb", [P, 16, 8], I8)
        mt = [sb(f"mt{i}", [P, 16, 8], BF) for i in range(2)]
        scr = sb("scr", [P, 2, 256], FP8)
        ps = [ctx.enter_context(nc.psum_tensor(f"ps{i}", [P, KC], F32))
              for i in range(4)]
        psd = ctx.enter_context(nc.psum_tensor("psd", [P, 256], F32))

        sem = lambda nm: ctx.enter_context(nc.semaphore(nm))
        sin0, sym = sem("sin0"), sem("sym")
        sin1a, sin1b = sem("sin1a"), sem("sin1b")
        spsA, spsB = sem("spsA"), sem("spsB")
        sev0, sevA, sevB = sem("sev0"), sem("sevA"), sem("sevB")
        smini, sscr = sem("smini"), sem("sscr")
        st_sp = sem("st_sp")
        st_gp = sem("st_gp")
        block = ctx.enter_context(nc.Block())

        we = ydw0[:, :, 0:OC]
        wo = ydw0[:, :, P:P + OC]
        t0e = lambda c: ydw0[:, :, 2 * P + c * KC:2 * P + (c + 1) * KC]
        t0o = lambda c: ydw0[:, :, 2 * P + HP + c * KC:2 * P + HP + (c + 1) * KC]

        @block.sync
        def _(sync):
            sync.dma_start(out=ydw0[:], in_=ydw0d[:]).then_inc(sin0, 16)
            sync.dma_start(out=yms[:], in_=ymd[:]).then_inc(sym, 16)
            sync.dma_start(out=ys1[:, :, :, 0:KC],
                           in_=yd1d[:, :, :, 0:KC]).then_inc(sin1a, 16)
            sync.dma_start(out=ys1[:, :, :, KC:HP],
                           in_=yd1d[:, :, :, KC:HP]).then_inc(sin1b, 16)
            sync.dma_start(out=e1d[:, KC:HP], in_=es[1][0:OC, KC:HP])._wait_ge(
                sevB, 1).then_inc(st_sp, 16)
            sync.wait_ge(st_sp, 32)

        @block.tensor
        def _(pe):
            dummy = lambda: pe.matmul(psd[0:P, :], scr[:, :, 0:P],
                                      scr[:, :, 0:256], start=True, stop=True,
                                      perf_mode=DR)
            small = lambda: pe.matmul(psd[0:32, 0:64], scr[:, :, 0:32],
                                      scr[:, :, 0:64], start=True, stop=True,
                                      perf_mode=DR)
            dummy()._wait_ge(sscr, 1)
            for _ in range(NWARM - 1):
                dummy()
            pe.wait_ge(sin0, 16)
            for c in range(2):
                pe.matmul(ps[c][0:OC, :], we, t0e(c), start=True,
                          stop=False, perf_mode=DR)
                pe.matmul(ps[c][0:OC, :], wo, t0o(c), start=False, stop=True,
                          perf_mode=DR).then_inc(spsA, 1)
            for _ in range(NGAP1):
                small()
            pe.wait_ge(sin1a, 16)
            pe.matmul(ps[2][0:OC, :], we, ys1[:, :, 0, 0:KC], start=True,
                      stop=False, perf_mode=DR)
            pe.matmul(ps[2][0:OC, :], wo, ys1[:, :, 1, 0:KC], start=False,
                      stop=True, perf_mode=DR).then_inc(spsB, 1)
            for _ in range(NGAP2):
                small()
            pe.wait_ge(sin1b, 16)
            pe.matmul(ps[3][0:OC, :], we, ys1[:, :, 0, KC:HP], start=True,
                      stop=False, perf_mode=DR)
            pe.matmul(ps[3][0:OC, :], wo, ys1[:, :, 1, KC:HP], start=False,
                      stop=True, perf_mode=DR).then_inc(spsB, 1)

        @block.scalar
        def _(act):
            act.activation(es[0][0:OC, 0:KC], ps[0][0:OC, :], Copy,
                           bias=-32768.0, scale=256.0)._wait_ge(
                spsA, 1).then_inc(sev0, 1)
            act.activation(es[1][0:OC, 0:KC], ps[2][0:OC, :], Copy,
                           bias=-32768.0, scale=256.0)._wait_ge(
                spsB, 1).then_inc(sevA, 1)
            act.dma_start(out=e1d[:, 0:KC], in_=es[1][0:OC, 0:KC])._wait_ge(
                sevA, 1).then_inc(st_sp, 16)

        @block.vector
        def _(v):
            v.memset(scr[:], 0).then_inc(sscr, 1)
            Pm, Qm, Rm = yms[:, 0], yms[:, 1], yms[:, 2]
            # mini: E = P[j]+P[j-1]+Q[j-2]+R[j-3]+R[j-4] - 128
            v.tensor_tensor(mt[0][:], Pm[:, :, 4:12], Pm[:, :, 3:11],
                            _A.add)._wait_ge(sym, 16)
            v.tensor_tensor(mt[1][:], Qm[:, :, 2:10], Rm[:, :, 1:9], _A.add)
            v.tensor_tensor(mt[0][:], mt[0][:], Rm[:, :, 0:8], _A.add)
            v.tensor_tensor(mt[0][:], mt[0][:], mt[1][:], _A.add)
            v.tensor_scalar(ems[:], mt[0][:], -128.0, None,
                            _A.add).then_inc(smini, 1)
            # evacuations c1 of each tile
            v.tensor_scalar(es[0][0:OC, KC:HP], ps[1][0:OC, :], 256.0,
                            -32768.0, _A.mult, _A.add)._wait_ge(
                spsA, 2).then_inc(sev0, 1)
            v.tensor_scalar(es[1][0:OC, KC:HP], ps[3][0:OC, :],
                            256.0, -32768.0, _A.mult, _A.add)._wait_ge(
                spsB, 2).then_inc(sevB, 1)

        @block.gpsimd
        def _(gp):
            gp.dma_start(out=emd[:], in_=ems[:])._wait_ge(
                smini, 1).then_inc(st_gp, 16)
            gp.dma_start(out=e0d[:], in_=es[0][0:OC, :])._wait_ge(
                sev0, 2).then_inc(st_gp, 16)
            gp.wait_ge(st_gp, 32)

    return nc


def _host_inputs(stm):
    f8 = ml_dtypes.float8_e4m3
    bf = ml_dtypes.bfloat16
    stm2d = np.asarray(stm, dtype=np.int32).reshape(R, R)      # [r, c]
    T = np.ascontiguousarray(stm2d.T)                          # [c, r]
    y2 = (T == 2).astype(np.float32)
    y3x = ((T == 3) + 6.0 * (T == 1)).astype(np.float32)
    # split row parities: [c, parity, HP]
    y2p = y2.reshape(R, HP, 2).transpose(0, 2, 1)
    y3p = y3x.reshape(R, HP, 2).transpose(0, 2, 1)
    padz = np.zeros((4, 2, HP), np.float32)
    y2p = np.concatenate([padz, y2p], axis=0).astype(f8)       # c index +4
    y3p = np.concatenate([padz, y3p], axis=0).astype(f8)

    w = np.zeros((P, 2, 2 * P), np.float32)
    for j in range(OC):
        w[j + 4, 0, j] = 2.0                # W2 band: taps (2, 2, 1)
        w[j + 3, 0, j] = 2.0
        w[j + 2, 0, j] = 1.0
        w[j:j + 5, 1, j] = 6.0              # W3 band: 6 * ones(5)
    w[:, :, P:] = w[:, :, 0:P] * (1.0 / 256.0)                 # odd weights
    wf8 = w.astype(f8)

    in_maps = []
    for k in range(N_CORES):
        B = CP * k
        # ydw0 data region: per slot, even columns then odd columns
        s0 = np.concatenate([y2p[B:B + P, 0], y2p[B:B + P, 1]], axis=-1)
        s1 = np.concatenate([y3p[B:B + P, 0], y3p[B:B + P, 1]], axis=-1)
        t0 = np.stack([s0, s1], axis=1)                        # [P, 2, 2HP]
        ydw0 = np.concatenate([wf8, t0], axis=2)               # [P, 2, 2304]
        t1 = np.stack([y2p[B + OC:B + OC + P],
                       y3p[B + OC:B + OC + P]], axis=1)        # [P, 2, 2, HP]
        # mini: columns B+244..B+256 (4 halo + 8 out), row-major
        sl = stm2d[:, B + 244:B + 256]                         # [r, 12]
        m2 = (sl == 2).astype(np.float32)
        y3 = ((sl == 3) + 6.0 * (sl == 1)).astype(np.float32)
        Pw, Qw, Rw = 2 * m2 + 6 * y3, m2 + 6 * y3, 6 * y3
        ym = np.stack([Pw, Qw, Rw], axis=1)                    # [r, 3, 12]
        ym = ym.reshape(16, P, 3, 12).transpose(1, 2, 0, 3)    # [P,3,16,12]
        in_maps.append({
            "ydw0": np.ascontiguousarray(ydw0),
            "yd1": np.ascontiguousarray(t1),
            "ym": np.ascontiguousarray(ym.astype(bf)),
        })
    return in_maps


def kernel(site_type_map, node_size_x, node_size_y, width, height,
           num_bins_x, num_bins_y, xl, xh, yl, yh):
    global LAST_RESULTS
    nc = _build_program()
    in_maps = _host_inputs(site_type_map)
    res = run_bass_kernel_spmd(nc, in_maps, core_ids=list(range(N_CORES)))
    LAST_RESULTS = res

    ET = np.empty((R, R), np.int16)        # [c, r], E in 0..215
    for k in range(N_CORES):
        B = CP * k
        rk = res.results[k]
        for t, nm in ((0, "e0"), (1, "e1")):
            e = np.asarray(rk[nm]).astype(np.int32) + 32768    # [OC, HP]
            blk = ET[B + t * OC:B + (t + 1) * OC]
            blk[:, 0::2] = e >> 8
            blk[:, 1::2] = e & 255
        em = np.asarray(rk["em"]).astype(np.int16) + 128       # [P, 16, 8]
        ET[B + 248:B + 256] = em.transpose(1, 0, 2).reshape(R, 8).T

    c2x2 = ET % 6
    c3 = (ET // 6) % 6
    g1 = (ET // 36).astype(np.int8)
    # m1[c] = g1[c] - g1[c-1] + m1[c-5]: stride-5 cumsum along c of diff(g1)
    d = np.empty((R + 2, R), np.int8)      # pad c-length 2048 -> 2050
    d[0] = g1[0]
    np.subtract(g1[1:], g1[:-1], out=d[1:R])
    d[R:] = 0
    m1 = np.cumsum(d.reshape(410, 5, R), axis=0, dtype=np.int16)
    m1 = m1.reshape(R + 2, R)[:R]

    out0 = np.ascontiguousarray((1.0 - m1).astype(np.float32).T)
    out2 = np.ascontiguousarray((1.0 - 0.5 * c2x2).astype(np.float32).T)
    out3 = np.ascontiguousarray((1.0 - c3).astype(np.float32).T)
    return (out0, out0, out2, out3)


# revision 27
# speedup vs baseline: 1.0049x; 1.0022x over previous
"""DemandMap (histogram_binning) Trainium2 Bass kernel — packed-pair encode.

Math (binW=binH=1, integer sites, sx=1): per row r, along c:
  cap1[c] = m1[c];  cap2[c] = m2[c] + m2[c-1] + 0.5 m2[c-2];
  cap3[c] = sum_{s<5} m3[c-s];  out_t = 1 - cap_t (out0 == out1).

Per site the device computes E = c2x2 + 6*c3 + 36*g1 (c2x2 = 2*cap2 via
taps (2,2,1) on y2 = m2; c3 via 6*ones(5) taps on y3x = m3 + 6*m1 whose
m1 rider gives g1 = 5-tap m1 sum; base-6 fields, E <= 215). Each fp8
DoubleRow matmul (k-slot 0 = W2 band on y2, k-slot 1 = W3 band on y3x;
0.5 cyc/row, K-independent) handles ONE row parity; even and odd rows
accumulate into the same PSUM column with the odd WEIGHTS scaled 2^-8
(per-matmul sums stay same-scale -> reduction tree exact; the f32 PSUM
accumulator adds E_even + E_odd/256 exactly). One evacuation per chunk
converts x256 - 32768 to int16 = 256*E_even + E_odd - 32768: two sites
per evacuated element, so evac free-size halves and stores stay 1 B/site.

Host decode: +32768 -> (hi, lo) = (E_even, E_odd); c2x2 = E%6 and
c3 = (E//6)%6 are the device rasterizations; m1 (1x1 sites, cap1 is
just the mask) via a stride-5 cumsum of diff(g1 = E//36).

Column-sharded: 2 x 124-col column-major tiles per core + an 8-col
row-major bf16 mini path (5 DVE ops). PE is kept continuously busy with
scratch warmup matmuls (plus standalone SEQ-level waits, so matmul
costs are evaluated at a late dispatch time = full clock). Stores issue
from three different sequencers (SP/ACT/Pool) to overlap their waits.
"""

from contextlib import ExitStack

import numpy as np
import ml_dtypes

import concourse.bass as bass
import concourse.mybir as mybir
from concourse.bass_utils import run_bass_kernel_spmd

N_CORES = 8
R = 2048              # rows
HP = 1024             # row pairs
CP = 256              # output columns per core
P = 128               # partitions
OC = 124              # output columns per main tile
KC = 512              # matmul chunk (one PSUM bank; 512 row-pairs)

_A = mybir.AluOpType
BF = mybir.dt.bfloat16
FP8 = mybir.dt.float8e4
I8 = mybir.dt.int8
I16 = mybir.dt.int16
F32 = mybir.dt.float32
Copy = mybir.ActivationFunctionType.Copy
DR = mybir.MatmulPerfMode.DoubleRow

NWARM, NGAP1, NGAP2, NGAP3 = 29, 44, 11, 19    # PE warmup / gap-filler matmuls

LAST_RESULTS = None


def _build_program():
    nc = bass.Bass()
    # [P, slot(y2/y3x), 128 Weven | 128 Wodd | 1024 even | 1024 odd]
    ydw0d = nc.dram_tensor("ydw0", [P, 2, 2 * P + 2 * HP], FP8,
                           kind="ExternalInput")
    yd1ad = nc.dram_tensor("yd1a", [P, 2, 2, KC], FP8, kind="ExternalInput")
    yd1bd = nc.dram_tensor("yd1b", [P, 2, 2, 256], FP8, kind="ExternalInput")
    yd1cd = nc.dram_tensor("yd1c", [P, 2, 2, 256], FP8, kind="ExternalInput")
    ymd = nc.dram_tensor("ym", [P, 3, 16, 12], BF, kind="ExternalInput")
    e0d = nc.dram_tensor("e0", [OC, HP], I16, kind="ExternalOutput")
    e1d = nc.dram_tensor("e1", [OC, HP], I16, kind="ExternalOutput")
    emd = nc.dram_tensor("em", [P, 16, 8], I8, kind="ExternalOutput")

    with ExitStack() as ctx:
        sb = lambda nm, shape, dt: ctx.enter_context(nc.sbuf_tensor(nm, shape, dt))
        ydw0 = sb("ydw0s", [P, 2, 2 * P + 2 * HP], FP8)
        ys1a = sb("ys1as", [P, 2, 2, KC], FP8)
        ys1b = sb("ys1bs", [P, 2, 2, 256], FP8)
        ys1c = sb("ys1cs", [P, 2, 2, 256], FP8)
        yms = sb("ymsb", [P, 3, 16, 12], BF)
        es = [sb(f"es{t}", [P, HP], I16) for t in range(2)]
        ems = sb("emsb", [P, 16, 8], I8)
        mt = [sb(f"mt{i}", [P, 16, 8], BF) for i in range(2)]
        scr = sb("scr", [P, 2, 256], FP8)
        ps = [ctx.enter_context(nc.psum_tensor(f"ps{i}", [P, KC], F32))
              for i in range(4)]
        psd = ctx.enter_context(nc.psum_tensor("psd", [P, 256], F32))

        sem = lambda nm: ctx.enter_context(nc.semaphore(nm))
        sin0, sym = sem("sin0"), sem("sym")
        sin1a, sin1b, sin1c = sem("sin1a"), sem("sin1b"), sem("sin1c")
        spsA, spsB = sem("spsA"), sem("spsB")
        sev0, sevA, sevB = sem("sev0"), sem("sevA"), sem("sevB")
        smini, sscr = sem("smini"), sem("sscr")
        st_sp = sem("st_sp")
        st_gp = sem("st_gp")
        block = ctx.enter_context(nc.Block())

        we = ydw0[:, :, 0:OC]
        wo = ydw0[:, :, P:P + OC]
        t0e = lambda c: ydw0[:, :, 2 * P + c * KC:2 * P + (c + 1) * KC]
        t0o = lambda c: ydw0[:, :, 2 * P + HP + c * KC:2 * P + HP + (c + 1) * KC]

        @block.sync
        def _(sync):
            sync.dma_start(out=ydw0[:], in_=ydw0d[:]).then_inc(sin0, 16)
            sync.dma_start(out=yms[:], in_=ymd[:]).then_inc(sym, 16)
            sync.dma_start(out=ys1a[:], in_=yd1ad[:]).then_inc(sin1a, 16)
            sync.dma_start(out=ys1b[:], in_=yd1bd[:]).then_inc(sin1b, 16)
            sync.dma_start(out=ys1c[:], in_=yd1cd[:]).then_inc(sin1c, 16)
            sync.dma_start(out=e1d[:, KC:HP], in_=es[1][0:OC, KC:HP])._wait_ge(
                sevB, 2).then_inc(st_sp, 16)
            sync.wait_ge(st_sp, 32)

        @block.tensor
        def _(pe):
            dummy = lambda: pe.matmul(psd[0:P, :], scr[:, :, 0:P],
                                      scr[:, :, 0:256], start=True, stop=True,
                                      perf_mode=DR)
            small = lambda: pe.matmul(psd[0:32, 0:64], scr[:, :, 0:32],
                                      scr[:, :, 0:64], start=True, stop=True,
                                      perf_mode=DR)
            dummy()._wait_ge(sscr, 1)
            for _ in range(NWARM - 1):
                dummy()
            pe.wait_ge(sin0, 16)
            for c in range(2):
                pe.matmul(ps[c][0:OC, :], we, t0e(c), start=True,
                          stop=False, perf_mode=DR)
                pe.matmul(ps[c][0:OC, :], wo, t0o(c), start=False, stop=True,
                          perf_mode=DR).then_inc(spsA, 1)
            for _ in range(NGAP1):
                small()
            pe.wait_ge(sin1a, 16)
            pe.matmul(ps[2][0:OC, :], we, ys1a[:, :, 0, :], start=True,
                      stop=False, perf_mode=DR)
            pe.matmul(ps[2][0:OC, :], wo, ys1a[:, :, 1, :], start=False,
                      stop=True, perf_mode=DR).then_inc(spsB, 1)
            for _ in range(NGAP2):
                small()
            pe.wait_ge(sin1b, 16)
            pe.matmul(ps[3][0:OC, 0:256], we, ys1b[:, :, 0, :], start=True,
                      stop=False, perf_mode=DR)
            pe.matmul(ps[3][0:OC, 0:256], wo, ys1b[:, :, 1, :], start=False,
                      stop=True, perf_mode=DR).then_inc(spsB, 1)
            for _ in range(NGAP3):
                small()
            pe.wait_ge(sin1c, 16)
            pe.matmul(ps[3][0:OC, 256:KC], we, ys1c[:, :, 0, :], start=True,
                      stop=False, perf_mode=DR)
            pe.matmul(ps[3][0:OC, 256:KC], wo, ys1c[:, :, 1, :], start=False,
                      stop=True, perf_mode=DR).then_inc(spsB, 1)

        @block.scalar
        def _(act):
            act.activation(es[0][0:OC, 0:KC], ps[0][0:OC, :], Copy,
                           bias=-32768.0, scale=256.0)._wait_ge(
                spsA, 1).then_inc(sev0, 1)
            act.activation(es[1][0:OC, 0:KC], ps[2][0:OC, :], Copy,
                           bias=-32768.0, scale=256.0)._wait_ge(
                spsB, 1).then_inc(sevA, 1)
            act.dma_start(out=e1d[:, 0:KC], in_=es[1][0:OC, 0:KC])._wait_ge(
                sevA, 1).then_inc(st_sp, 16)

        @block.vector
        def _(v):
            v.memset(scr[:], 0).then_inc(sscr, 1)
            Pm, Qm, Rm = yms[:, 0], yms[:, 1], yms[:, 2]
            # mini: E = P[j]+P[j-1]+Q[j-2]+R[j-3]+R[j-4] - 128
            v.tensor_tensor(mt[0][:], Pm[:, :, 4:12], Pm[:, :, 3:11],
                            _A.add)._wait_ge(sym, 16)
            v.tensor_tensor(mt[1][:], Qm[:, :, 2:10], Rm[:, :, 1:9], _A.add)
            v.tensor_tensor(mt[0][:], mt[0][:], Rm[:, :, 0:8], _A.add)
            v.tensor_tensor(mt[0][:], mt[0][:], mt[1][:], _A.add)
            v.tensor_scalar(ems[:], mt[0][:], -128.0, None,
                            _A.add).then_inc(smini, 1)
            # evacuations c1 of each tile
            v.tensor_scalar(es[0][0:OC, KC:HP], ps[1][0:OC, :], 256.0,
                            -32768.0, _A.mult, _A.add)._wait_ge(
                spsA, 2).then_inc(sev0, 1)
            v.tensor_scalar(es[1][0:OC, KC:KC + KC // 2], ps[3][0:OC, 0:256],
                            256.0, -32768.0, _A.mult, _A.add)._wait_ge(
                spsB, 2).then_inc(sevB, 1)
            v.tensor_scalar(es[1][0:OC, KC + KC // 2:HP], ps[3][0:OC, 256:KC],
                            256.0, -32768.0, _A.mult, _A.add)._wait_ge(
                spsB, 3).then_inc(sevB, 1)

        @block.gpsimd
        def _(gp):
            gp.dma_start(out=e0d[:], in_=es[0][0:OC, :])._wait_ge(
                sev0, 2).then_inc(st_gp, 16)
            gp.dma_start(out=emd[:], in_=ems[:])._wait_ge(
                smini, 1).then_inc(st_gp, 16)
            gp.wait_ge(st_gp, 32)

    return nc


def _host_inputs(stm):
    f8 = ml_dtypes.float8_e4m3
    bf = ml_dtypes.bfloat16
    stm2d = np.asarray(stm, dtype=np.int32).reshape(R, R)      # [r, c]
    T = np.ascontiguousarray(stm2d.T)                          # [c, r]
    y2 = (T == 2).astype(np.float32)
    y3x = ((T == 3) + 6.0 * (T == 1)).astype(np.float32)
    # split row parities: [c, parity, HP]
    y2p = y2.reshape(R, HP, 2).transpose(0, 2, 1)
    y3p = y3x.reshape(R, HP, 2).transpose(0, 2, 1)
    padz = np.zeros((4, 2, HP), np.float32)
    y2p = np.concatenate([padz, y2p], axis=0).astype(f8)       # c index +4
    y3p = np.concatenate([padz, y3p], axis=0).astype(f8)

    w = np.zeros((P, 2, 2 * P), np.float32)
    for j in range(OC):
        w[j + 4, 0, j] = 2.0                # W2 band: taps (2, 2, 1)
        w[j + 3, 0, j] = 2.0
        w[j + 2, 0, j] = 1.0
        w[j:j + 5, 1, j] = 6.0              # W3 band: 6 * ones(5)
    w[:, :, P:] = w[:, :, 0:P] * (1.0 / 256.0)                 # odd weights
    wf8 = w.astype(f8)

    in_maps = []
    for k in range(N_CORES):
        B = CP * k
        # ydw0 data region: per slot, even columns then odd columns
        s0 = np.concatenate([y2p[B:B + P, 0], y2p[B:B + P, 1]], axis=-1)
        s1 = np.concatenate([y3p[B:B + P, 0], y3p[B:B + P, 1]], axis=-1)
        t0 = np.stack([s0, s1], axis=1)                        # [P, 2, 2HP]
        ydw0 = np.concatenate([wf8, t0], axis=2)               # [P, 2, 2304]
        t1 = np.stack([y2p[B + OC:B + OC + P],
                       y3p[B + OC:B + OC + P]], axis=1)        # [P, 2, 2, HP]
        t1a = np.ascontiguousarray(t1[:, :, :, 0:KC])
        t1b = np.ascontiguousarray(t1[:, :, :, KC:768])
        t1c = np.ascontiguousarray(t1[:, :, :, 768:1024])
        # mini: columns B+244..B+256 (4 halo + 8 out), row-major
        sl = stm2d[:, B + 244:B + 256]                         # [r, 12]
        m2 = (sl == 2).astype(np.float32)
        y3 = ((sl == 3) + 6.0 * (sl == 1)).astype(np.float32)
        Pw, Qw, Rw = 2 * m2 + 6 * y3, m2 + 6 * y3, 6 * y3
        ym = np.stack([Pw, Qw, Rw], axis=1)                    # [r, 3, 12]
        ym = ym.reshape(16, P, 3, 12).transpose(1, 2, 0, 3)    # [P,3,16,12]
        in_maps.append({
            "ydw0": np.ascontiguousarray(ydw0),
            "yd1a": t1a,
            "yd1b": t1b,
            "yd1c": t1c,
            "ym": np.ascontiguousarray(ym.astype(bf)),
        })
    return in_maps


def kernel(site_type_map, node_size_x, node_size_y, width, height,
           num_bins_x, num_bins_y, xl, xh, yl, yh):
    global LAST_RESULTS
    nc = _build_program()
    in_maps = _host_inputs(site_type_map)
    res = run_bass_kernel_spmd(nc, in_maps, core_ids=list(range(N_CORES)))
    LAST_RESULTS = res

    ET = np.empty((R, R), np.int16)        # [c, r], E in 0..215
    for k in range(N_CORES):
        B = CP * k
        rk = res.results[k]
        for t, nm in ((0, "e0"), (1, "e1")):
            e = np.asarray(rk[nm]).astype(np.int32) + 32768    # [OC, HP]
            blk = ET[B + t * OC:B + (t + 1) * OC]
            blk[:, 0::2] = e >> 8
            blk[:, 1::2] = e & 255
        em = np.asarray(rk["em"]).astype(np.int16) + 128       # [P, 16, 8]
        ET[B + 248:B + 256] = em.transpose(1, 0, 2).reshape(R, 8).T

    c2x2 = ET % 6
    c3 = (ET // 6) % 6
    g1 = (ET // 36).astype(np.int8)
    # m1[c] = g1[c] - g1[c-1] + m1[c-5]: stride-5 cumsum along c of diff(g1)
    d = np.empty((R + 2, R), np.int8)      # pad c-length 2048 -> 2050
    d[0] = g1[0]
    np.subtract(g1[1:], g1[:-1], out=d[1:R])
    d[R:] = 0
    m1 = np.cumsum(d.reshape(410, 5, R), axis=0, dtype=np.int16)
    m1 = m1.reshape(R + 2, R)[:R]

    out0 = np.ascontiguousarray((1.0 - m1).astype(np.float32).T)
    out2 = np.ascontiguousarray((1.0 - 0.5 * c2x2).astype(np.float32).T)
    out3 = np.ascontiguousarray((1.0 - c3).astype(np.float32).T)
    return (out0, out0, out2, out3)


# revision 28
# speedup vs baseline: 1.0503x; 1.0452x over previous
"""DemandMap (histogram_binning) Trainium2 Bass kernel — packed-pair encode.

Math (binW=binH=1, integer sites, sx=1): per row r, along c:
  cap1[c] = m1[c];  cap2[c] = m2[c] + m2[c-1] + 0.5 m2[c-2];
  cap3[c] = sum_{s<5} m3[c-s];  out_t = 1 - cap_t (out0 == out1).

Per site the device computes E = c2x2 + 6*c3 + 36*g1 (c2x2 = 2*cap2 via
taps (2,2,1) on y2 = m2; c3 via 6*ones(5) taps on y3x = m3 + 6*m1 whose
m1 rider gives g1 = 5-tap m1 sum; base-6 fields, E <= 215). Each fp8
DoubleRow matmul (k-slot 0 = W2 band on y2, k-slot 1 = W3 band on y3x;
0.5 cyc/row, K-independent) handles ONE row parity; even and odd rows
accumulate into the same PSUM column with the odd WEIGHTS scaled 2^-8
(per-matmul sums stay same-scale -> reduction tree exact; the f32 PSUM
accumulator adds E_even + E_odd/256 exactly). One evacuation per chunk
converts x256 - 32768 to int16 = 256*E_even + E_odd - 32768: two sites
per evacuated element, so evac free-size halves and stores stay 1 B/site.

Host decode: +32768 -> (hi, lo) = (E_even, E_odd); c2x2 = E%6 and
c3 = (E//6)%6 are the device rasterizations; m1 (1x1 sites, cap1 is
just the mask) via a stride-5 cumsum of diff(g1 = E//36).

Column-sharded: 2 x 124-col column-major tiles per core + an 8-col
row-major bf16 mini path (5 DVE ops). PE is kept continuously busy with
scratch warmup matmuls (plus standalone SEQ-level waits, so matmul
costs are evaluated at a late dispatch time = full clock). Stores issue
from three different sequencers (SP/ACT/Pool) to overlap their waits.
"""

from contextlib import ExitStack

import numpy as np
import ml_dtypes

import concourse.bass as bass
import concourse.mybir as mybir
from concourse.bass_utils import run_bass_kernel_spmd

N_CORES = 8
R = 2048              # rows
HP = 1024             # row pairs
CP = 256              # output columns per core
P = 128               # partitions
OC = 124              # output columns per main tile
KC = 512              # matmul chunk (one PSUM bank; 512 row-pairs)

_A = mybir.AluOpType
BF = mybir.dt.bfloat16
FP8 = mybir.dt.float8e4
I8 = mybir.dt.int8
I16 = mybir.dt.int16
F32 = mybir.dt.float32
Copy = mybir.ActivationFunctionType.Copy
DR = mybir.MatmulPerfMode.DoubleRow

NWARM, NGAP1, NGAP2, NGAP3 = 29, 44, 11, 19    # PE warmup / gap-filler matmuls

LAST_RESULTS = None


def _build_program():
    nc = bass.Bass()
    # [P, slot(y2/y3x), 128 Weven | 128 Wodd | 1024 even | 1024 odd]
    ydw0d = nc.dram_tensor("ydw0", [P, 2, 2 * P + 2 * HP], FP8,
                           kind="ExternalInput")
    yd1ad = nc.dram_tensor("yd1a", [P, 2, 2, KC], FP8, kind="ExternalInput")
    yd1bd = nc.dram_tensor("yd1b", [P, 2, 2, 256], FP8, kind="ExternalInput")
    yd1cd = nc.dram_tensor("yd1c", [P, 2, 2, 256], FP8, kind="ExternalInput")
    ymd = nc.dram_tensor("ym", [P, 3, 16, 12], BF, kind="ExternalInput")
    e0d = nc.dram_tensor("e0", [OC, HP], I16, kind="ExternalOutput")
    e1d = nc.dram_tensor("e1", [OC, HP], I16, kind="ExternalOutput")
    emd = nc.dram_tensor("em", [P, 16, 8], I8, kind="ExternalOutput")

    with ExitStack() as ctx:
        sb = lambda nm, shape, dt: ctx.enter_context(nc.sbuf_tensor(nm, shape, dt))
        ydw0 = sb("ydw0s", [P, 2, 2 * P + 2 * HP], FP8)
        ys1a = sb("ys1as", [P, 2, 2, KC], FP8)
        ys1b = sb("ys1bs", [P, 2, 2, 256], FP8)
        ys1c = sb("ys1cs", [P, 2, 2, 256], FP8)
        yms = sb("ymsb", [P, 3, 16, 12], BF)
        es = [sb(f"es{t}", [P, HP], I16) for t in range(2)]
        ems = sb("emsb", [P, 16, 8], I8)
        mt = [sb(f"mt{i}", [P, 16, 8], BF) for i in range(2)]
        scr = sb("scr", [P, 2, 256], FP8)
        ps = [ctx.enter_context(nc.psum_tensor(f"ps{i}", [P, KC], F32))
              for i in range(4)]
        psd = ctx.enter_context(nc.psum_tensor("psd", [P, 256], F32))

        sem = lambda nm: ctx.enter_context(nc.semaphore(nm))
        sin0, sym = sem("sin0"), sem("sym")
        sin1a, sin1b, sin1c = sem("sin1a"), sem("sin1b"), sem("sin1c")
        spsA, spsB = sem("spsA"), sem("spsB")
        sev0, sevA, sevB = sem("sev0"), sem("sevA"), sem("sevB")
        smini, sscr = sem("smini"), sem("sscr")
        st_sp = sem("st_sp")
        st_gp = sem("st_gp")
        block = ctx.enter_context(nc.Block())

        we = ydw0[:, :, 0:OC]
        wo = ydw0[:, :, P:P + OC]
        t0e = lambda c: ydw0[:, :, 2 * P + c * KC:2 * P + (c + 1) * KC]
        t0o = lambda c: ydw0[:, :, 2 * P + HP + c * KC:2 * P + HP + (c + 1) * KC]

        @block.sync
        def _(sync):
            sync.dma_start(out=ydw0[:], in_=ydw0d[:]).then_inc(sin0, 16)
            sync.dma_start(out=yms[:], in_=ymd[:]).then_inc(sym, 16)
            sync.dma_start(out=ys1a[:], in_=yd1ad[:]).then_inc(sin1a, 16)
            sync.dma_start(out=ys1b[:], in_=yd1bd[:]).then_inc(sin1b, 16)
            sync.dma_start(out=ys1c[:], in_=yd1cd[:]).then_inc(sin1c, 16)
            sync.dma_start(out=e1d[:, KC:HP], in_=es[1][0:OC, KC:HP])._wait_ge(
                sevB, 2).then_inc(st_sp, 16)
            sync.wait_ge(st_sp, 32)

        @block.tensor
        def _(pe):
            dummy = lambda: pe.matmul(psd[0:P, :], scr[:, :, 0:P],
                                      scr[:, :, 0:256], start=True, stop=True,
                                      perf_mode=DR)
            small = lambda: pe.matmul(psd[0:32, 0:64], scr[:, :, 0:32],
                                      scr[:, :, 0:64], start=True, stop=True,
                                      perf_mode=DR)
            dummy()._wait_ge(sscr, 1)
            for _ in range(NWARM - 1):
                dummy()
            pe.wait_ge(sin0, 16)
            for c in range(2):
                pe.matmul(ps[c][0:OC, :], we, t0e(c), start=True,
                          stop=False, perf_mode=DR)
                pe.matmul(ps[c][0:OC, :], wo, t0o(c), start=False, stop=True,
                          perf_mode=DR).then_inc(spsA, 1)
            for _ in range(NGAP1):
                small()
            pe.wait_ge(sin1a, 16)
            pe.matmul(ps[2][0:OC, :], we, ys1a[:, :, 0, :], start=True,
                      stop=False, perf_mode=DR)
            pe.matmul(ps[2][0:OC, :], wo, ys1a[:, :, 1, :], start=False,
                      stop=True, perf_mode=DR).then_inc(spsB, 1)
            for _ in range(NGAP2):
                small()
            pe.wait_ge(sin1b, 16)
            pe.matmul(ps[3][0:OC, 0:256], we, ys1b[:, :, 0, :], start=True,
                      stop=False, perf_mode=DR)
            pe.matmul(ps[3][0:OC, 0:256], wo, ys1b[:, :, 1, :], start=False,
                      stop=True, perf_mode=DR).then_inc(spsB, 1)
            for _ in range(NGAP3):
                small()
            pe.wait_ge(sin1c, 16)
            pe.matmul(ps[3][0:OC, 256:KC], we, ys1c[:, :, 0, :], start=True,
                      stop=False, perf_mode=DR)
            pe.matmul(ps[3][0:OC, 256:KC], wo, ys1c[:, :, 1, :], start=False,
                      stop=True, perf_mode=DR).then_inc(spsB, 1)

        @block.scalar
        def _(act):
            act.activation(es[0][0:OC, 0:KC], ps[0][0:OC, :], Copy,
                           bias=-32768.0, scale=256.0)._wait_ge(
                spsA, 1).then_inc(sev0, 1)
            act.activation(es[1][0:OC, 0:KC], ps[2][0:OC, :], Copy,
                           bias=-32768.0, scale=256.0)._wait_ge(
                spsB, 1).then_inc(sevA, 1)
            act.dma_start(out=e1d[:, 0:KC], in_=es[1][0:OC, 0:KC])._wait_ge(
                sevA, 1).then_inc(st_sp, 16)

        @block.vector
        def _(v):
            v.memset(scr[:], 0).then_inc(sscr, 1)
            Pm, Qm, Rm = yms[:, 0], yms[:, 1], yms[:, 2]
            # tile0 c1 evacuation first: it gates the big e0 store
            v.tensor_scalar(es[0][0:OC, KC:HP], ps[1][0:OC, :], 256.0,
                            -32768.0, _A.mult, _A.add)._wait_ge(
                spsA, 2).then_inc(sev0, 1)
            # mini: E = P[j]+P[j-1]+Q[j-2]+R[j-3]+R[j-4] - 128
            v.tensor_tensor(mt[0][:], Pm[:, :, 4:12], Pm[:, :, 3:11],
                            _A.add)._wait_ge(sym, 16)
            v.tensor_tensor(mt[1][:], Qm[:, :, 2:10], Rm[:, :, 1:9], _A.add)
            v.tensor_tensor(mt[0][:], mt[0][:], Rm[:, :, 0:8], _A.add)
            v.tensor_tensor(mt[0][:], mt[0][:], mt[1][:], _A.add)
            v.tensor_scalar(ems[:], mt[0][:], -128.0, None,
                            _A.add).then_inc(smini, 1)
            v.tensor_scalar(es[1][0:OC, KC:KC + KC // 2], ps[3][0:OC, 0:256],
                            256.0, -32768.0, _A.mult, _A.add)._wait_ge(
                spsB, 2).then_inc(sevB, 1)
            v.tensor_scalar(es[1][0:OC, KC + KC // 2:HP], ps[3][0:OC, 256:KC],
                            256.0, -32768.0, _A.mult, _A.add)._wait_ge(
                spsB, 3).then_inc(sevB, 1)

        @block.gpsimd
        def _(gp):
            gp.dma_start(out=e0d[:], in_=es[0][0:OC, :])._wait_ge(
                sev0, 2).then_inc(st_gp, 16)
            gp.dma_start(out=emd[:], in_=ems[:])._wait_ge(
                smini, 1).then_inc(st_gp, 16)
            gp.wait_ge(st_gp, 32)

    return nc


def _host_inputs(stm):
    f8 = ml_dtypes.float8_e4m3
    bf = ml_dtypes.bfloat16
    stm2d = np.asarray(stm, dtype=np.int32).reshape(R, R)      # [r, c]
    T = np.ascontiguousarray(stm2d.T)                          # [c, r]
    y2 = (T == 2).astype(np.float32)
    y3x = ((T == 3) + 6.0 * (T == 1)).astype(np.float32)
    # split row parities: [c, parity, HP]
    y2p = y2.reshape(R, HP, 2).transpose(0, 2, 1)
    y3p = y3x.reshape(R, HP, 2).transpose(0, 2, 1)
    padz = np.zeros((4, 2, HP), np.float32)
    y2p = np.concatenate([padz, y2p], axis=0).astype(f8)       # c index +4
    y3p = np.concatenate([padz, y3p], axis=0).astype(f8)

    w = np.zeros((P, 2, 2 * P), np.float32)
    for j in range(OC):
        w[j + 4, 0, j] = 2.0                # W2 band: taps (2, 2, 1)
        w[j + 3, 0, j] = 2.0
        w[j + 2, 0, j] = 1.0
        w[j:j + 5, 1, j] = 6.0              # W3 band: 6 * ones(5)
    w[:, :, P:] = w[:, :, 0:P] * (1.0 / 256.0)                 # odd weights
    wf8 = w.astype(f8)

    in_maps = []
    for k in range(N_CORES):
        B = CP * k
        # ydw0 data region: per slot, even columns then odd columns
        s0 = np.concatenate([y2p[B:B + P, 0], y2p[B:B + P, 1]], axis=-1)
        s1 = np.concatenate([y3p[B:B + P, 0], y3p[B:B + P, 1]], axis=-1)
        t0 = np.stack([s0, s1], axis=1)                        # [P, 2, 2HP]
        ydw0 = np.concatenate([wf8, t0], axis=2)               # [P, 2, 2304]
        t1 = np.stack([y2p[B + OC:B + OC + P],
                       y3p[B + OC:B + OC + P]], axis=1)        # [P, 2, 2, HP]
        t1a = np.ascontiguousarray(t1[:, :, :, 0:KC])
        t1b = np.ascontiguousarray(t1[:, :, :, KC:768])
        t1c = np.ascontiguousarray(t1[:, :, :, 768:1024])
        # mini: columns B+244..B+256 (4 halo + 8 out), row-major
        sl = stm2d[:, B + 244:B + 256]                         # [r, 12]
        m2 = (sl == 2).astype(np.float32)
        y3 = ((sl == 3) + 6.0 * (sl == 1)).astype(np.float32)
        Pw, Qw, Rw = 2 * m2 + 6 * y3, m2 + 6 * y3, 6 * y3
        ym = np.stack([Pw, Qw, Rw], axis=1)                    # [r, 3, 12]
        ym = ym.reshape(16, P, 3, 12).transpose(1, 2, 0, 3)    # [P,3,16,12]
        in_maps.append({
            "ydw0": np.ascontiguousarray(ydw0),
            "yd1a": t1a,
            "yd1b": t1b,
            "yd1c": t1c,
            "ym": np.ascontiguousarray(ym.astype(bf)),
        })
    return in_maps


def kernel(site_type_map, node_size_x, node_size_y, width, height,
           num_bins_x, num_bins_y, xl, xh, yl, yh):
    global LAST_RESULTS
    nc = _build_program()
    in_maps = _host_inputs(site_type_map)
    res = run_bass_kernel_spmd(nc, in_maps, core_ids=list(range(N_CORES)))
    LAST_RESULTS = res

    ET = np.empty((R, R), np.int16)        # [c, r], E in 0..215
    for k in range(N_CORES):
        B = CP * k
        rk = res.results[k]
        for t, nm in ((0, "e0"), (1, "e1")):
            e = np.asarray(rk[nm]).astype(np.int32) + 32768    # [OC, HP]
            blk = ET[B + t * OC:B + (t + 1) * OC]
            blk[:, 0::2] = e >> 8
            blk[:, 1::2] = e & 255
        em = np.asarray(rk["em"]).astype(np.int16) + 128       # [P, 16, 8]
        ET[B + 248:B + 256] = em.transpose(1, 0, 2).reshape(R, 8).T

    c2x2 = ET % 6
    c3 = (ET // 6) % 6
    g1 = (ET // 36).astype(np.int8)
    # m1[c] = g1[c] - g1[c-1] + m1[c-5]: stride-5 cumsum along c of diff(g1)
    d = np.empty((R + 2, R), np.int8)      # pad c-length 2048 -> 2050
    d[0] = g1[0]
    np.subtract(g1[1:], g1[:-1], out=d[1:R])
    d[R:] = 0
    m1 = np.cumsum(d.reshape(410, 5, R), axis=0, dtype=np.int16)
    m1 = m1.reshape(R + 2, R)[:R]

    out0 = np.ascontiguousarray((1.0 - m1).astype(np.float32).T)
    out2 = np.ascontiguousarray((1.0 - 0.5 * c2x2).astype(np.float32).T)
    out3 = np.ascontiguousarray((1.0 - c3).astype(np.float32).T)
    return (out0, out0, out2, out3)
